# revision 2
# baseline (speedup 1.0000x reference)
"""Trainium2 Bass kernel for nn_Discriminator (histogram_binning / ridge).

Math (reference):
  For each batch n (N=32): interpolate P=128 points into M=(P-1)*181=22987
  line points (x,y,w); splat Gaussians gx[m,s]=exp(-(x_m-s)^2/(2 w_m)),
  gy[m,t]; canvas = gx^T @ gy [128,128]; line = tanh(canvas);
  loss = sum(BCE(line, img))/N + sum(poly_sqrt(seg_len^2))/N.

Strategy (data-parallel over N, 4 batches/core, 8 cores, ONE SPMD program):
  Line points are Hilbert-sorted so chunks of 128 consecutive points are
  spatially local; Gaussians are evaluated only on a square WxW window
  (W in {32,64,96,128}, the union window across the 8 cores at the same
  chunk rank, x-side 32-aligned).  Up to 8 chunks of the same W-class are
  stacked into ONE K=128 arg matmul: chunk s occupies coefficient rows
  16s..16s+15 of the stationary and a 2W-column block of a constant
  block-diagonal basis table (so LDWEIGHTS and the ~30ns/MM issue cost
  amortize 8x, and K=128 keeps the PE's HAM clock-gate warm).  Exp args
  are produced in bf16-bit-space (t = 128/ln2 * arg + B) so exp splits
  between ScalarE (true Exp via the free scale/bias affine) and DVE
  (fast-exp: i16 convert of max(t,0), bitcast bf16, ~3% max err).  The
  canvas accumulates in PSUM via 1-2 windowed matmul pieces per chunk
  (output partition bases restricted to {0,32,64}).  The BCE epilogue
  runs on Pool/DVE inline per batch; the Ln phase is deferred to the end
  so the ACT table set switches only once.
"""
import os
import sys
import types
import numpy as np
import ml_dtypes

# ---------------------------------------------------------------- constants
IMG = 128
P = 128
N = 32
CMP = int(IMG * np.sqrt(2))            # 181
M = (P - 1) * CMP                      # 22987
CH = 128
NCHUNK = (M + CH - 1) // CH            # 180
MPAD = NCHUNK * CH                     # 23040
NCORES = 8
NB = N // NCORES                       # 4
_os = os
KAPPA = float(_os.environ.get("KN_KAPPA", "10.0"))
USE_XHALF = _os.environ.get("KN_XHALF", "1") == "1"
LN2 = float(np.log(2.0))
K2 = 128.0 / LN2
MAGIC = 5.5
BBIAS = 16256.0 - MAGIC
ACT_SHARE = float(_os.environ.get("KN_ACT", "0.55"))   # fraction of exp groups on ScalarE
WYCLASSES = (32, 64, 96, 128)
WXCLASSES = (64, 128)
WCLASSES = [(wx, wy) for wx in WXCLASSES for wy in WYCLASSES]
WCAP = {(wx, wy): min(5, 1024 // (wx + wy)) for (wx, wy) in WCLASSES}
ROWS_PER_SLOT = 22                     # (3,2,3,3) split rows x 2 axes

_d = np.arange(-IMG + 1, IMG)
X0 = float((_d ** 2 + (_d ** 2).T).mean().astype(np.float32))
C0 = float(X0 ** 0.5)
C1 = float(X0 ** (-0.5) / 2.0)
C2 = float(-(X0 ** (-1.5) / 8.0))
C3 = float(X0 ** (-2.5) / 16.0)

_BF = ml_dtypes.bfloat16

TANH_SAT = float(np.uint32(1090516548).view(np.float32))  # 7.9988117
ULP_BELOW_1 = 5.960464477539063e-08


def _install_ntff_hook():
    if 'antenv.axon_hooks' in sys.modules:
        return
    mod = types.ModuleType('antenv.axon_hooks')
    _h = [None]
    mod.set_axon_ntff_profile_hook = lambda h: _h.__setitem__(0, h)
    mod.get_axon_ntff_profile_hook = lambda: _h[0]
    sys.modules['antenv.axon_hooks'] = mod
    try:
        from trn_agent_boot.trn_boot import _ntff_profile_via_ctypes
        mod.set_axon_ntff_profile_hook(
            _ntff_profile_via_ctypes('/opt/axon/libaxon_pjrt.so'))
    except Exception:
        pass


_install_ntff_hook()

import concourse.bass as bass          # noqa: E402
import concourse.tile as tile          # noqa: E402
from concourse import bacc, mybir      # noqa: E402
from concourse.bass_utils import run_bass_kernel_spmd  # noqa: E402

dt = mybir.dt
AF = mybir.ActivationFunctionType
ALU = mybir.AluOpType


# ---------------------------------------------------------------- host prep
def _hilbert_d(xc, yc, n):
    x_, y_ = xc.copy(), yc.copy()
    d = np.zeros_like(xc)
    s = n // 2
    while s > 0:
        rx = ((x_ & s) > 0).astype(np.int64)
        ry = ((y_ & s) > 0).astype(np.int64)
        d += s * s * ((3 * rx) ^ ry)
        swap = ry == 0
        xr = np.where(swap & (rx == 1), s - 1 - x_, x_)
        yr = np.where(swap & (rx == 1), s - 1 - y_, y_)
        x_, y_ = np.where(swap, yr, xr), np.where(swap, xr, yr)
        s //= 2
    return d


def _interp_sorted(points):
    pts = np.asarray(points, np.float64)
    t = (np.arange(CMP, dtype=np.float64) / CMP)[None, None, :, None]
    lp = ((1.0 - t) * pts[:, :-1, None, :]
          + t * pts[:, 1:, None, :]).reshape(N, M, 3)
    xs = np.zeros((N, MPAD)); ys = np.zeros((N, MPAD)); ws = np.ones((N, MPAD))
    dead = np.zeros((N, MPAD), bool)
    dead[:, M:] = True
    for n in range(N):
        x, y, w = lp[n, :, 0], lp[n, :, 1], lp[n, :, 2]
        hx = np.clip(x.astype(np.int64), 0, IMG - 1)
        hy = np.clip(y.astype(np.int64), 0, IMG - 1)
        o = np.argsort(_hilbert_d(hx, hy, IMG), kind='stable')
        xs[n, :M], ys[n, :M], ws[n, :M] = x[o], y[o], w[o]
    return xs, ys, ws, dead


def _tight_windows(xs, ys, ws, dead):
    out = np.zeros((N, NCHUNK, 4), np.int64)
    for n in range(N):
        xc = xs[n].reshape(NCHUNK, CH)
        yc = ys[n].reshape(NCHUNK, CH)
        wc = ws[n].reshape(NCHUNK, CH)
        live = ~dead[n].reshape(NCHUNK, CH)
        wmax = np.maximum(np.where(live, wc, 0.0).max(1), 0.5)
        r = KAPPA * np.sqrt(wmax)
        xl = np.where(live, xc, np.inf).min(1)
        xh = np.where(live, xc, -np.inf).max(1)
        yl = np.where(live, yc, np.inf).min(1)
        yh = np.where(live, yc, -np.inf).max(1)
        alldead = ~live.any(1)
        xl = np.where(alldead, 0.0, xl); xh = np.where(alldead, 0.0, xh)
        yl = np.where(alldead, 0.0, yl); yh = np.where(alldead, 0.0, yh)
        out[n, :, 0] = np.maximum(np.floor(xl - r), 0)
        out[n, :, 1] = np.minimum(np.ceil(xh + r), IMG - 1)
        out[n, :, 2] = np.maximum(np.floor(yl - r), 0)
        out[n, :, 3] = np.minimum(np.ceil(yh + r), IMG - 1)
    return out


def _build_schedule(points):
    xs, ys, ws, dead = _interp_sorted(points)
    tw = _tight_windows(xs, ys, ws, dead)
    slots = []                                  # [NB][NCHUNK] dicts
    groups = []                                 # [NB][ngroups]
    for b in range(NB):
        rows = []
        for r in range(NCHUNK):
            u = tw[[k * NB + b for k in range(NCORES)], r]
            xlo, xhi = int(u[:, 0].min()), int(u[:, 1].max())
            ylo, yhi = int(u[:, 2].min()), int(u[:, 3].max())
            wyt = yhi - ylo + 1
            W = min(32 * int(np.ceil(wyt / 32)), 128)
            oy = max(0, min(ylo - (W - wyt) // 2, IMG - W))
            if USE_XHALF and xhi < 64:
                wx, xoff = 64, 0
            elif USE_XHALF and xlo >= 64:
                wx, xoff = 64, 64
            else:
                wx, xoff = 128, 0
            rows.append(dict(W=W, wx=wx, xoff=xoff, oy=oy, ylo=ylo, wyt=wyt))
        gs = []
        for (wx, wy) in WCLASSES:
            for xoff in ((0, 64) if wx == 64 else (0,)):
                rs = [r for r in range(NCHUNK)
                      if (rows[r]['wx'], rows[r]['xoff'],
                          rows[r]['W']) == (wx, xoff, wy)]
                cap = WCAP[(wx, wy)]
                for i in range(0, len(rs), cap):
                    gs.append(dict(W=wy, wx=wx, xoff=xoff,
                                   chunks=rs[i:i + cap]))
        for gi, g in enumerate(gs):
            for s, r in enumerate(g['chunks']):
                rows[r]['grp'] = gi
                rows[r]['slot'] = s
        slots.append(rows)
        groups.append(gs)
    return dict(slots=slots, groups=groups), (xs, ys, ws, dead)


def _split_bf(c, n):
    out = []
    r = np.asarray(c, np.float64).copy()
    for _ in range(n):
        h = r.astype(_BF).astype(np.float64)
        out.append(h)
        r = r - h
    return out


def _build_coeffs(sched, geom):
    """F [N, 128, NGMAX*128] bf16: group gi at column block gi; chunk at
    slot s occupies rows 22s..22s+22 (11 x-rows then 11 y-rows).
    Split layout per axis: c2 x3 (on v2h), c2 x2 (on v2l), c1 x3, c0 x3."""
    xs, ys, ws, dead = geom
    slots, groups = sched['slots'], sched['groups']
    ngmax = max(len(groups[b]) for b in range(NB))
    F = np.zeros((N, 128, ngmax * CH), _BF)
    for n in range(N):
        b = n % NB
        for r in range(NCHUNK):
            s = slots[b][r]
            W, gi, si = s['W'], s['grp'], s['slot']
            sl = slice(r * CH, (r + 1) * CH)
            xc, yc, wc, dd = xs[n, sl], ys[n, sl], ws[n, sl], dead[n, sl]
            invw = 1.0 / wc
            col = slice(gi * CH, gi * CH + CH)
            for ax, d_ in enumerate((xc - 64.0,
                                     yc - (s['oy'] + W // 2))):
                c2 = np.where(dd, 0.0, -0.5 * invw * K2)
                c1 = np.where(dd, 0.0, d_ * invw * K2)
                c0 = np.where(dd, -300000.0,
                              -0.5 * d_ * d_ * invw * K2 + BBIAS)
                rows = (_split_bf(c2, 3) + _split_bf(c2, 2)
                        + _split_bf(c1, 3) + _split_bf(c0, 3))
                base = ROWS_PER_SLOT * si + 11 * ax
                for j, rv in enumerate(rows):
                    F[n, base + j, col] = rv.astype(_BF)
    return F, ngmax


def _build_qtabs():
    """Constant basis tables per (wx, wy) class: q [128, cap*(wx+wy)] bf16.
    Slot s block: x-basis over [xoff? no -- both half-grids share d=x-64:
    the x columns are u in [0,wx) mapped to canvas cols (handled by which
    table: for wx=64 there are TWO variants (xoff 0 / 64)].  To keep one
    table per class, wx=64 tables carry BOTH half-grids is impossible --
    instead xoff is baked per-class: classes are (wx, xoff?, wy).  We use
    separate table entries keyed (wx, xoff, wy) built lazily below."""
    tabs = {}
    for (wx, wy) in WCLASSES:
        for xoff in ((0, 64) if wx == 64 else (0,)):
            cap = WCAP[(wx, wy)]
            blk = wx + wy
            q = np.zeros((128, cap * blk), _BF)
            for ax, (wid, u0) in enumerate(((wx, xoff), (wy, 64 - wy // 2))):
                v = np.arange(u0, u0 + wid, dtype=np.float64) - 64.0
                v2 = v * v
                v2h = v2.astype(_BF).astype(np.float64)
                v2l = v2 - v2h
                ones = np.ones(wid)
                rows = [v2h, v2h, v2h, v2l, v2l, v, v, v, ones, ones, ones]
                for s in range(cap):
                    c0_ = s * blk + (0 if ax == 0 else wx)
                    for j, rv in enumerate(rows):
                        q[ROWS_PER_SLOT * s + 11 * ax + j,
                          c0_: c0_ + wid] = rv.astype(_BF)
            tabs[(wx, xoff, wy)] = q
    return tabs


# ---------------------------------------------------------------- device
def _exp_pattern(n):
    pat = []
    acc = 0.0
    for _ in range(n):
        acc += ACT_SHARE
        if acc >= 1.0:
            pat.append('act')
            acc -= 1.0
        else:
            pat.append('dve')
    return pat


def _build_nc(sched, ngmax):
    slots, groups = sched['slots'], sched['groups']
    qtabs = _build_qtabs()
    qkeys = sorted(qtabs.keys())
    qoff = {}
    off = 0
    for k in qkeys:
        qoff[k] = off
        off += qtabs[k].shape[1]
    qtotal = off

    nc = bacc.Bacc("TRN2", target_bir_lowering=False, debug=False,
                   enable_asserts=False, num_devices=NCORES)
    f_in = nc.dram_tensor("fco", [NB, 128, ngmax * CH], dt.bfloat16,
                          kind="ExternalInput").ap()
    q_in = nc.dram_tensor("qtab", [128, qtotal], dt.bfloat16,
                          kind="ExternalInput").ap()
    img_in = nc.dram_tensor("img", [NB, IMG, IMG], dt.float32,
                            kind="ExternalInput").ap()
    ptsa_in = nc.dram_tensor("ptsa", [NB, P - 1, 2], dt.float32,
                             kind="ExternalInput").ap()
    ptsb_in = nc.dram_tensor("ptsb", [NB, P - 1, 2], dt.float32,
                             kind="ExternalInput").ap()
    out = nc.dram_tensor("out", [128, 2 * NB], dt.float32,
                         kind="ExternalOutput").ap()

    with tile.TileContext(nc) as tc:
        with tc.tile_pool(name="const", bufs=1) as const_pool, \
             tc.tile_pool(name="fpool", bufs=2) as fpool, \
             tc.tile_pool(name="gpool", bufs=3) as gpool, \
             tc.tile_pool(name="small", bufs=2) as small, \
             tc.tile_pool(name="epi", bufs=2) as epi, \
             tc.tile_pool(name="canvps", bufs=1, space="PSUM") as canvps, \
             tc.tile_pool(name="argps", bufs=3, space="PSUM") as argps:

            qt = const_pool.tile([128, qtotal], dt.bfloat16)
            nc.sync.dma_start(qt[:], q_in[:])
            outsb = const_pool.tile([128, 2 * NB], dt.float32)
            nc.vector.memset(outsb[:], 0.0)
            m100 = const_pool.tile([128, NB * IMG], dt.float32)
            nc.vector.memset(m100[:], -100.0)
            mant_mask = const_pool.tile([128, 1], dt.int32)
            nc.vector.memset(mant_mask[:], 0x007FFFFF)
            one_bits = const_pool.tile([128, 1], dt.int32)
            nc.vector.memset(one_bits[:], 0x3F800000)
            expbias = const_pool.tile([128, 1], dt.float32)
            nc.vector.memset(expbias[:], -BBIAS * (LN2 / 128.0))
            canvas = canvps.tile([128, NB * IMG], dt.float32)
            nc.vector.memset(canvas[:], 0.0)

            imgt4 = const_pool.tile([128, NB * IMG], dt.float32)
            for b in range(NB):
                gs = groups[b]
                ng = len(gs)
                pat = _exp_pattern(ng)
                ft = fpool.tile([128, ngmax * CH], dt.bfloat16, name="ft")
                nsl = 8
                wsl = (ng * CH) // nsl
                for s in range(nsl):
                    lo = s * wsl
                    hi = (s + 1) * wsl if s < nsl - 1 else ng * CH
                    nc.sync.dma_start(ft[:, lo:hi], f_in[b][:, lo:hi])
                nc.sync.dma_start(imgt4[:, b * IMG:(b + 1) * IMG], img_in[b])

                gtiles = []
                for gi, g in enumerate(gs):
                    qk = (g['wx'], g['xoff'], g['W'])
                    n2w = len(g['chunks']) * (g['wx'] + g['W'])
                    at = argps.tile([128, 1024], dt.float32, name="argt")
                    for a in range(0, n2w, 512):
                        bnd = min(a + 512, n2w)
                        nc.tensor.matmul(
                            at[:, a:bnd],
                            ft[:, gi * CH:(gi + 1) * CH],
                            qt[:, qoff[qk] + a: qoff[qk] + bnd],
                            start=True, stop=True)
                    gt = gpool.tile([128, 1024], dt.bfloat16, name="gt")
                    if pat[gi] == 'act':
                        nc.scalar.activation(gt[:, 0:n2w], at[:, 0:n2w],
                                             AF.Exp,
                                             bias=expbias[:, 0:1],
                                             scale=(LN2 / 128.0))
                    else:
                        nc.vector.tensor_scalar(
                            gt[:, 0:n2w].bitcast(dt.int16),
                            at[:, 0:n2w], 0.0, None, ALU.max)
                    gtiles.append(gt)
                    if gi >= 2:
                        _canvas_mms(nc, canvas, b, slots[b], gs[gi - 2],
                                    gtiles[gi - 2])
                for gi in (len(gs) - 2, len(gs) - 1):
                    if gi >= 0:
                        _canvas_mms(nc, canvas, b, slots[b], gs[gi],
                                    gtiles[gi])

                # ---- distance term (small; Pool + one DVE reduce)
                ta = small.tile([P - 1, 2], dt.float32, name="ta")
                tb = small.tile([P - 1, 2], dt.float32, name="tb")
                nc.sync.dma_start(ta[:], ptsa_in[b])
                nc.sync.dma_start(tb[:], ptsb_in[b])
                dxy = epi.tile([P - 1, 2], dt.float32, name="dxy")
                nc.gpsimd.tensor_tensor(dxy[:], tb[:], ta[:], ALU.subtract)
                nc.gpsimd.tensor_tensor(dxy[:], dxy[:], dxy[:], ALU.mult)
                segsq = epi.tile([P - 1, 1], dt.float32, name="segsq")
                nc.vector.tensor_reduce(segsq[:], dxy[:],
                                        mybir.AxisListType.X, ALU.add)
                dx = epi.tile([P - 1, 1], dt.float32, name="dx")
                nc.gpsimd.tensor_scalar(dx[:], segsq[:], -X0, None, ALU.add)
                poly = epi.tile([P - 1, 1], dt.float32, name="poly")
                nc.gpsimd.tensor_scalar(poly[:], dx[:], C3, C2,
                                        ALU.mult, ALU.add)
                nc.gpsimd.tensor_tensor(poly[:], poly[:], dx[:], ALU.mult)
                nc.gpsimd.tensor_scalar(poly[:], poly[:], C1, None, ALU.add)
                nc.gpsimd.tensor_tensor(poly[:], poly[:], dx[:], ALU.mult)
                nc.gpsimd.tensor_scalar(outsb[:P - 1, NB + b:NB + b + 1],
                                        poly[:], C0, None, ALU.add)

            # ---- deferred epilogue, batched [128, NB*IMG]
            CW = NB * IMG
            line4 = epi.tile([128, CW], dt.float32, name="line4")
            nc.scalar.activation(line4[:], canvas[:], AF.Tanh)
            msat4 = epi.tile([128, CW], dt.uint8, name="msat4")
            nc.vector.tensor_scalar(msat4[:], canvas[:], TANH_SAT, None,
                                    ALU.is_ge)
            mb4 = epi.tile([128, CW], dt.int32, name="mb4")
            nc.vector.tensor_scalar(mb4[:], line4[:].bitcast(dt.int32),
                                    mant_mask[:, 0:1], one_bits[:, 0:1],
                                    ALU.bitwise_and, ALU.bitwise_or)
            u4 = epi.tile([128, CW], dt.float32, name="u4")
            nc.vector.tensor_scalar(u4[:], line4[:], -1.0, 1.0,
                                    ALU.mult, ALU.add)
            nc.vector.tensor_scalar(u4[:], u4[:], ULP_BELOW_1, None, ALU.max)
            db4 = epi.tile([128, CW], dt.int32, name="db4")
            nc.vector.tensor_tensor(db4[:], line4[:].bitcast(dt.int32),
                                    mb4[:], ALU.subtract)
            ef4 = epi.tile([128, CW], dt.float32, name="ef4")
            nc.vector.tensor_copy(ef4[:], db4[:])
            nc.vector.tensor_scalar(ef4[:], ef4[:],
                                    0.6931471805599453 / (1 << 23),
                                    None, ALU.mult)
            logp4 = epi.tile([128, CW], dt.float32, name="logp4")
            nc.scalar.activation(logp4[:], mb4[:].bitcast(dt.float32), AF.Ln)
            nc.vector.tensor_tensor(logp4[:], logp4[:], ef4[:], ALU.add)
            mlow4 = epi.tile([128, CW], dt.uint8, name="mlow4")
            nc.vector.tensor_scalar(mlow4[:], line4[:], 1e-38, None,
                                    ALU.is_lt)
            nc.vector.copy_predicated(logp4[:], mlow4[:], m100[:])
            log1mp4 = epi.tile([128, CW], dt.float32, name="log1mp4")
            nc.scalar.activation(log1mp4[:], u4[:], AF.Ln)
            nc.vector.copy_predicated(log1mp4[:], msat4[:], m100[:])
            diff4 = epi.tile([128, CW], dt.float32, name="diff4")
            nc.vector.tensor_tensor(diff4[:], logp4[:], log1mp4[:],
                                    ALU.subtract)
            prod4 = epi.tile([128, CW], dt.float32, name="prod4")
            nc.vector.tensor_tensor(prod4[:], imgt4[:], diff4[:], ALU.mult)
            tot4 = epi.tile([128, CW], dt.float32, name="tot4")
            nc.vector.tensor_tensor(tot4[:], prod4[:], log1mp4[:], ALU.add)
            for b in range(NB):
                nc.vector.tensor_reduce(outsb[:, b:b + 1],
                                        tot4[:, b * IMG:(b + 1) * IMG],
                                        mybir.AxisListType.X, ALU.add)
            nc.sync.dma_start(out[:], outsb[:])
    import os
    if os.environ.get("SKIP_COMPILE") != "1":
        nc.compile()
    return nc


def _canvas_mms(nc, canvas, b, bslots, g, gt):
    wx, W = g['wx'], g['W']
    blk_w = wx + W
    for si, r in enumerate(g['chunks']):
        s = bslots[r]
        blk = si * blk_w
        gx = gt[:, blk: blk + wx]
        gy = gt[:, blk + wx + (s['ylo'] - s['oy']):
                blk + wx + (s['ylo'] - s['oy']) + s['wyt']]
        nc.tensor.matmul(
            canvas[s['xoff']: s['xoff'] + wx,
                   b * IMG + s['ylo']: b * IMG + s['ylo'] + s['wyt']],
            gx, gy, start=False, stop=False, skip_group_check=True)


_NC_CACHE = {}


def make_in_maps(points, img, sched, geom, ngmax):
    points = np.asarray(points, np.float32)
    img = np.asarray(img, np.float32)
    F, _ = _build_coeffs(sched, geom)
    qtabs = _build_qtabs()
    q = np.concatenate([qtabs[k] for k in sorted(qtabs.keys())], axis=1)
    in_maps = []
    for c in range(NCORES):
        sl = slice(c * NB, (c + 1) * NB)
        pts = points[sl]
        in_maps.append({
            "fco": np.ascontiguousarray(F[sl]),
            "qtab": q,
            "img": np.ascontiguousarray(img[sl]),
            "ptsa": np.ascontiguousarray(pts[:, :P - 1, 0:2]),
            "ptsb": np.ascontiguousarray(pts[:, 1:, 0:2]),
        })
    return in_maps


def combine_outputs(results):
    bce_tot = 0.0
    dist_tot = 0.0
    for r in results:
        o = np.asarray(r["out"], np.float64)
        bce_tot += o[:, :NB].sum()
        dist_tot += o[:P - 1, NB:].sum()
    return np.float32((dist_tot - bce_tot) / N)


def kernel(points, img, _trace=False, _trace_kwargs=None):
    sched, geom = _build_schedule(points)
    ngmax = max(len(sched['groups'][b]) for b in range(NB))
    key = repr([(g['W'], tuple(g['chunks'])) for b in range(NB)
                for g in sched['groups'][b]]) + repr(
        [(s['W'], s['wx'], s['xoff'], s['oy'], s['ylo'], s['wyt'])
         for b in range(NB) for s in sched['slots'][b]])
    import hashlib
    key = hashlib.sha1(key.encode()).hexdigest()
    if key not in _NC_CACHE:
        _NC_CACHE.clear()
        _NC_CACHE[key] = _build_nc(sched, ngmax)
    nc = _NC_CACHE[key]
    in_maps = make_in_maps(points, img, sched, geom, ngmax)
    kw = {}
    if _trace:
        kw.update(trace=True, trace_cores=[0])
        if _trace_kwargs:
            kw.update(_trace_kwargs)
    res = run_bass_kernel_spmd(nc, in_maps, core_ids=list(range(NCORES)), **kw)
    outv = combine_outputs(res.results)
    if _trace:
        return outv, res
    return outv
